# revision 77
# baseline (speedup 1.0000x reference)
"""Trainium2 Bass kernel for causal self-attention with RoPE (tensor-parallel over 8 cores).

Contract: kernel(**inputs) takes full unsharded inputs (x, W_attn, b_attn,
W_proj, b_proj), shards across 8 NeuronCores (2 heads each), runs one SPMD
Bass/Tile kernel, and host-reduces the partial c_proj outputs.

Optimizations over the 452us baseline (-> ~361us, rel err 7.8e-4):
  - fp16 datapath end-to-end (same PE rate as bf16, 4x finer mantissa).
  - softmax denominators off the PE's per-block ones-matmuls: DVE/gpsimd
    accumulate e-tiles in fp16, then ONE all-ones [128x128] matmul
    partition-reduces AND replicates z across partitions in a single
    512-cycle op (~50us less PE work than per-block [1,512] reductions).
  - normalize is a single DVE multiply reading yps straight from PSUM.
  - RoPE uses one rotate-half matmul; the cos term folds into the DVE
    evacuation add (halves rope PE time).
  - causal diagonal blocks use narrowed matmul free dims + a single 128x128
    triangular corner mask instead of full-width mask multiplies.
  - cold start streams 8 open psum accumulators over the contraction dim
    while wqk chunks + split xT strips land round-robin on 3 DMA queues.
  - attention(h0/h1) and c_proj are interleaved per query tile (c_proj
    lagging one tile, last group deferred into the next batch's QKV) so
    the scalar exp stream and the z/normalize chains never stall the PE.
  - queue discipline: out-DMAs own the sync queue (osb ring reuse waits on
    them); the next batch's xT prefetch rides gpsimd in two bursts.
"""

import os
import sys

import numpy as np

for _p in ("/opt/trn_rl_repo",):
    if os.path.isdir(_p) and _p not in sys.path:
        sys.path.insert(0, _p)

from contextlib import ExitStack

import concourse.bass as bass
import concourse.tile as tile
from concourse import bacc, mybir
from concourse.bass_utils import run_bass_kernel_spmd

# ---- problem constants (hardcoded per contract) ----
B, T, C = 2, 2048, 2048
H, D = 16, 128
N_CORES = 8
HPC = H // N_CORES  # heads per core = 2
ROPE_BASE = 10000.0
SCALE = float(1.0 / np.sqrt(D))
EXPB = -2.0         # exp bias: softmax-invariant, keeps e and z small for fp16
TQ = 512            # query tile (free dim of scores matmul)
NTQ = T // TQ       # 4
TK = 128            # key tile (partition dim of scoresT)
NTK = T // TK       # 16
NCT = C // 128      # 16 contraction tiles for projections
BT = B * T

F32 = mybir.dt.float32
F16 = mybir.dt.float16

ADD = mybir.AluOpType.add
MULT = mybir.AluOpType.mult
EXP = mybir.ActivationFunctionType.Exp

PAIR_LOOKAHEAD = 4  # score-pairs ahead of attV in the attention pipeline


def _build_program(with_bias_qk: bool, with_bias_v: bool):
    nc = bacc.Bacc(
        "TRN2", target_bir_lowering=False, debug=False, num_devices=N_CORES
    )

    xT = nc.dram_tensor("xT", [C, BT], F16, kind="ExternalInput").ap()
    wqk = nc.dram_tensor("wqk", [128, NCT, 4 * D], F16, kind="ExternalInput").ap()
    wv = nc.dram_tensor("wv", [128, NCT, HPC * D], F16, kind="ExternalInput").ap()
    wpr = nc.dram_tensor("wpr", [128, HPC, C], F16, kind="ExternalInput").ap()
    bqk = nc.dram_tensor("bqk", [128, 4], F32, kind="ExternalInput").ap()
    bv = nc.dram_tensor("bv", [HPC * D], F32, kind="ExternalInput").ap()
    cosT = nc.dram_tensor("cosT", [D, T], F16, kind="ExternalInput").ap()
    sinT = nc.dram_tensor("sinT", [D, T], F16, kind="ExternalInput").ap()
    tri = nc.dram_tensor("tri", [TK, TK], F16, kind="ExternalInput").ap()
    rmat = nc.dram_tensor("rmat", [D, D], F16, kind="ExternalInput").ap()
    out = nc.dram_tensor("out", [BT, C], F16, kind="ExternalOutput").ap()

    with tile.TileContext(nc) as tc, ExitStack() as ctx:
        consts = ctx.enter_context(tc.tile_pool(name="consts", bufs=1))
        xt_pool = ctx.enter_context(tc.tile_pool(name="xt", bufs=1))
        qk_pool = ctx.enter_context(tc.tile_pool(name="qk", bufs=1))
        v_pool = ctx.enter_context(tc.tile_pool(name="v", bufs=2))
        e_pool = ctx.enter_context(tc.tile_pool(name="e", bufs=6))
        sc_pool = ctx.enter_context(tc.tile_pool(name="sc", bufs=6))
        acc_pool = ctx.enter_context(tc.tile_pool(name="za", bufs=2))
        z_pool = ctx.enter_context(tc.tile_pool(name="zs", bufs=2))
        yn_pool = ctx.enter_context(tc.tile_pool(name="yn", bufs=1))
        ob_pool = ctx.enter_context(tc.tile_pool(name="ob", bufs=5))
        ps_a = ctx.enter_context(tc.tile_pool(name="ps_a", bufs=4, space="PSUM"))
        ps_b = ctx.enter_context(tc.tile_pool(name="ps_b", bufs=2, space="PSUM"))

        # ---- cold-start DMA: interleave wqk chunks with xT strips on 4
        # queues so the first qkv matmul group can start within a few us.
        ENGQ = [nc.sync, nc.gpsimd, nc.scalar]
        wqk_sb = consts.tile([128, NCT, 4 * D], F16)
        xt_sb = xt_pool.tile([128, NCT, T], F16, tag="xt")
        cos_sb = consts.tile([128, T], F16)
        sin_sb = consts.tile([128, T], F16)
        rmat_sb = consts.tile([128, D], F16)
        tri_sb = consts.tile([128, TK], F16)
        # pass 1 of the cold QKV only reads xT[:, 0:1024] of each strip, so
        # stream all first-halves (+ wqk) before any second-half: the
        # pass-1 critical DMA volume drops from 10MB to 6MB
        half = T // 2
        for ct in range(NCT):
            ENGQ[(2 * ct) % 3].dma_start(wqk_sb[:, ct, :], wqk[:, ct, :])
            ENGQ[(2 * ct + 1) % 3].dma_start(
                xt_sb[:, ct, 0:half], xT[ct * 128 : (ct + 1) * 128, 0:half]
            )
            if ct == 0:
                ENGQ[2].dma_start(rmat_sb[:], rmat[:])
                ENGQ[2].dma_start(tri_sb[:], tri[:])
        ENGQ[0].dma_start(cos_sb[:], cosT[:])
        ENGQ[1].dma_start(sin_sb[:], sinT[:])
        for ct in range(NCT):
            ENGQ[(2 * ct) % 3].dma_start(
                xt_sb[:, ct, half:T], xT[ct * 128 : (ct + 1) * 128, half:T]
            )

        expb_sb = consts.tile([128, 1], F32)
        nc.vector.memset(expb_sb[:], EXPB)
        ones_sb = consts.tile([128, 128], F16)
        nc.vector.memset(ones_sb[:], 1.0)
        wv_sb = consts.tile([128, NCT, HPC * D], F16)
        nc.sync.dma_start(wv_sb[:], wv[:])
        wpr_sb = consts.tile([128, HPC, C], F16)
        nc.gpsimd.dma_start(wpr_sb[:], wpr[:])
        if with_bias_qk:
            bqk_sb = consts.tile([128, 4], F32)
            nc.scalar.dma_start(bqk_sb[:], bqk[:])
        if with_bias_v:
            bv_sb = consts.tile([128, HPC * D], F32)
            nc.scalar.dma_start(bv_sb[:], bv.to_broadcast((128, HPC * D)))

        # qk feature tiles: 0=q_h0, 1=q_h1, 2=k_h0, 3=k_h1 (layout [D, T])
        def emit_rope_prep(qk_tiles, f, t, ps_ap):
            """DVE: qcos=(ps+b)*cos, qsin=(ps+b)*sin for one (f, t)."""
            tsl = slice(t * TQ, (t + 1) * TQ)
            qcos = sc_pool.tile([128, TQ], F16, tag="qcos")
            qsin = sc_pool.tile([128, TQ], F16, tag="qsin")
            bias_arg = bqk_sb[:, f : f + 1] if with_bias_qk else 0.0
            nc.vector.scalar_tensor_tensor(
                qcos[:], ps_ap, bias_arg, cos_sb[:, tsl], op0=ADD, op1=MULT
            )
            nc.vector.scalar_tensor_tensor(
                qsin[:], ps_ap, bias_arg, sin_sb[:, tsl], op0=ADD, op1=MULT
            )
            return (f, t, qcos, qsin)

        def emit_rope_mm(qk_tiles, f, t, qcos, qsin):
            """PE: rotate-half matmul; DVE: qk = rps + qcos (evac)."""
            tsl = slice(t * TQ, (t + 1) * TQ)
            rps = ps_a.tile([128, TQ], F32, tag="a", name=f"rp{f}{t}")
            nc.tensor.matmul(rps[:], rmat_sb[:], qsin[:], start=True, stop=True)
            with nc.allow_low_precision(reason="rope evac to fp16"):
                nc.vector.tensor_tensor(
                    qk_tiles[f][:, tsl], rps[:], qcos[:], op=ADD
                )

        def qkv_cold(xt_sb):
            """Batch-0 QKV: stream ct over 8 open accumulators (f x t01),
            then t2/t3 passes from SBUF-resident strips."""
            qk_tiles = [
                qk_pool.tile([128, T], F16, tag=f"qk{f}", name=f"qkt{f}")
                for f in range(4)
            ]
            accA = [
                ps_a.tile([128, TQ], F32, tag="a", name=f"csA{f}")
                for f in range(4)
            ]
            psb = [
                ps_b.tile([128, 2 * TQ], F32, tag="b", name=f"csB{i}")
                for i in range(2)
            ]
            accB = [
                psb[0][:, 0:TQ], psb[0][:, TQ:], psb[1][:, 0:TQ], psb[1][:, TQ:]
            ]
            for ct in range(NCT):
                for f in range(4):
                    nc.tensor.matmul(
                        accA[f][:],
                        wqk_sb[:, ct, f * D : (f + 1) * D],
                        xt_sb[:, ct, 0:TQ],
                        start=(ct == 0),
                        stop=(ct == NCT - 1),
                    )
                for f in range(4):
                    nc.tensor.matmul(
                        accB[f],
                        wqk_sb[:, ct, f * D : (f + 1) * D],
                        xt_sb[:, ct, TQ : 2 * TQ],
                        start=(ct == 0),
                        stop=(ct == NCT - 1),
                    )
            backlog = []

            def push_prep(prep):
                # keep <=4 preps outstanding: qcos/qsin rings have 6 bufs; a
                # deeper backlog would make a prep's WAR wait on an evac
                # emitted later in the DVE queue (in-order deadlock)
                backlog.append(prep)
                if len(backlog) > 4:
                    emit_rope_mm(qk_tiles, *backlog.pop(0))

            for f in range(4):
                push_prep(emit_rope_prep(qk_tiles, f, 0, accA[f][:]))
            for f in range(4):
                push_prep(emit_rope_prep(qk_tiles, f, 1, accB[f]))
            for t in (2, 3):
                for f in range(4):
                    ps = ps_a.tile([128, TQ], F32, tag="a")
                    for ct in range(NCT):
                        nc.tensor.matmul(
                            ps[:],
                            wqk_sb[:, ct, f * D : (f + 1) * D],
                            xt_sb[:, ct, t * TQ : (t + 1) * TQ],
                            start=(ct == 0),
                            stop=(ct == NCT - 1),
                        )
                    push_prep(emit_rope_prep(qk_tiles, f, t, ps[:]))
            return qk_tiles, backlog

        def qkv_warm(xt_sb, deferred=None):
            """Batch-1 QKV: strips already resident, plain f/t loops.

            `deferred` (emitted after the first mm group) carries the
            previous batch's last c_proj group so its normalize chain
            finishes while the PE works on this group.
            """
            qk_tiles = [
                qk_pool.tile([128, T], F16, tag=f"qk{f}", name=f"qkw{f}")
                for f in range(4)
            ]
            backlog = []
            for f in range(4):
                for t in range(NTQ):
                    ps = ps_a.tile([128, TQ], F32, tag="a")
                    for ct in range(NCT):
                        nc.tensor.matmul(
                            ps[:],
                            wqk_sb[:, ct, f * D : (f + 1) * D],
                            xt_sb[:, ct, t * TQ : (t + 1) * TQ],
                            start=(ct == 0),
                            stop=(ct == NCT - 1),
                        )
                    if deferred is not None:
                        deferred()
                        deferred = None
                    backlog.append(emit_rope_prep(qk_tiles, f, t, ps[:]))
                    if len(backlog) > 1:
                        emit_rope_mm(qk_tiles, *backlog.pop(0))
            return qk_tiles, backlog

        def v_phase(xt_sb, qk_tiles, backlog):
            """V in [t, d] layout: lhsT = xT tile (c, t), rhs = Wv (c, d).

            Drains the remaining rope backlog between v groups so the rope
            matmuls never wait back-to-back on their DVE preps.
            """
            v_sb = v_pool.tile([128, NTK, HPC * D], F16, tag="v")
            for mt in range(NTK):
                ps = ps_a.tile([128, HPC * D], F32, tag="a")
                for ct in range(NCT):
                    nc.tensor.matmul(
                        ps[:],
                        xt_sb[:, ct, mt * TK : (mt + 1) * TK],
                        wv_sb[:, ct, :],
                        start=(ct == 0),
                        stop=(ct == NCT - 1),
                    )
                if backlog:
                    emit_rope_mm(qk_tiles, *backlog.pop(0))
                if with_bias_v:
                    with nc.allow_low_precision(reason="v evac to fp16"):
                        nc.vector.tensor_add(v_sb[:, mt, :], ps[:], bv_sb[:])
                else:
                    # scalar engine: idle during qkv, and this keeps DVE
                    # clear for the attention phase that follows
                    nc.scalar.copy(v_sb[:, mt, :], ps[:])
            return v_sb

        def attn_core(hl, j, qk_tiles, v_sb, yn_sb):
            """One head x query tile: scores/exp/attV/z + normalize.

            Scores/attV/z use narrowed free dims on diagonal key blocks; only
            the 128x128 triangular corner needs a mask multiply. Softmax
            denominator: DVE accumulates e-tiles in fp16, then ONE all-ones
            [128x128] matmul partition-reduces accz AND replicates z across
            all partitions in a single 512-cycle PE op (free-dim bound, the
            wide out is free) -- no gpsimd partition_all_reduce latency.
            """
            qT = qk_tiles[hl]
            kT = qk_tiles[2 + hl]
            nblk = 4 * j + 4
            npair = nblk // 2
            yps = ps_a.tile([128, TQ], F32, tag="a", name=f"y{hl}{j}")
            accz = acc_pool.tile([128, TQ], F16, tag="acc")
            # Process the DIAGONAL pairs first: psum accumulation order is
            # commutative, and this way the exp->mask->attV chains of the
            # masked pairs complete with full pipeline slack while the tile
            # tail is far-key pairs whose exps are long since done.
            if j == 0:
                order = [0, 1]
            else:
                order = [2 * j, 2 * j + 1] + list(range(2 * j))
            first_blk = 2 * order[0]
            last_blk = 2 * order[-1] + 1
            # gpsimd takes the u2/u3 diagonal pair's masks + z partials (via
            # its own accumulator) for j>=2; at j<2 gpsimd is still busy
            # issuing the next batch's xT prefetch (SWDGE)
            gp_pair = npair - 1 if j >= 2 else -1
            if gp_pair >= 0:
                accg = acc_pool.tile([128, TQ], F16, tag="accg", name="accg")
            else:
                accg = None
            e_tiles = [None] * npair
            z_started = {"dve": False, "gp": False}

            def emit_pair(p):
                # two adjacent tk blocks share a 2-bank psum + one wide exp;
                # the final pair exps per half so its tail-latency is shorter
                sps = ps_b.tile([128, 2 * TQ], F32, tag="b")
                e = e_pool.tile([128, 2 * TQ], F16, tag="e")
                split = p == npair - 1
                for u in range(2):
                    i = 2 * p + u
                    co = TK * (i - 4 * j) if i >= 4 * j else 0
                    usl = slice(u * TQ + co, (u + 1) * TQ)
                    nc.tensor.matmul(
                        sps[:, usl],
                        kT[:, i * TK : (i + 1) * TK],
                        qT[:, j * TQ + co : (j + 1) * TQ],
                        start=True,
                        stop=True,
                    )
                    if split:
                        nc.scalar.activation(
                            e[:, usl], sps[:, usl], EXP,
                            bias=expb_sb[:], scale=SCALE,
                        )
                if not split:
                    nc.scalar.activation(
                        e[:], sps[:], EXP, bias=expb_sb[:], scale=SCALE
                    )
                e_tiles[p] = e

            def emit_consume(p):
                # z partial routing: gpsimd owns pair gp_pair via its own
                # accumulator, DVE accumulates the rest; each engine's first
                # processed block initializes its accumulator by copy
                e = e_tiles[p]
                gp = p == gp_pair
                eng = nc.gpsimd if gp else nc.vector
                acc = accg if gp else accz
                key = "gp" if gp else "dve"
                for u in range(2):
                    i = 2 * p + u
                    diag = i >= 4 * j
                    co = TK * (i - 4 * j) if diag else 0
                    if diag:
                        csl = slice(u * TQ + co, u * TQ + co + TK)
                        eng.tensor_mul(e[:, csl], e[:, csl], tri_sb[:])
                    eh = e[:, u * TQ + co : (u + 1) * TQ]
                    # z before attV: lets the z reduce overlap the trailing
                    # attV matmuls of the tile
                    with nc.allow_low_precision(reason="z fp16 partials"):
                        if not z_started[key]:
                            eng.tensor_copy(acc[:, co:TQ], eh)
                            z_started[key] = True
                        else:
                            eng.tensor_tensor(
                                acc[:, co:TQ], acc[:, co:TQ], eh, op=ADD
                            )
                    nc.tensor.matmul(
                        yps[:, co:TQ],
                        v_sb[:, i, hl * D : (hl + 1) * D],
                        eh,
                        start=(i == first_blk),
                        stop=(i == last_blk),
                        skip_group_check=True,
                    )

            for idx, p in enumerate(order):
                emit_pair(p)
                if idx >= PAIR_LOOKAHEAD:
                    emit_consume(order[idx - PAIR_LOOKAHEAD])
            for idx in range(max(0, npair - PAIR_LOOKAHEAD), npair):
                emit_consume(order[idx])

            jsl = slice(j * TQ, (j + 1) * TQ)
            zps = ps_a.tile([128, TQ], F32, tag="a", name=f"z{hl}{j}")
            nc.tensor.matmul(
                zps[:], ones_sb[:], accz[:], start=True, stop=(accg is None)
            )
            if accg is not None:
                # accg holds the u2/u3 diagonal pair: valid q-cols [256:512]
                nc.tensor.matmul(
                    zps[:, 2 * TK : TQ],
                    ones_sb[:],
                    accg[:, 2 * TK : TQ],
                    start=False,
                    stop=True,
                    skip_group_check=True,
                )
            zrec = z_pool.tile([128, TQ], F32, tag="zc")
            nc.vector.reciprocal_approx_fast(zrec[:], zps[:])
            with nc.allow_low_precision(reason="normalized y in fp16"):
                nc.vector.tensor_tensor(
                    yn_sb[:, jsl], yps[:], zrec[:], op=MULT
                )

        def emit_cproj_group(b, g, yn_h, qrot=False):
            """c_proj for the 4 token blocks of query tile g."""
            for m in range(4):
                mt = 4 * g + m
                osb = ob_pool.tile([128, C], F16, tag="ob")
                for n in range(NTQ):
                    ops = ps_a.tile([128, TQ], F32, tag="a")
                    for hl in range(HPC):
                        nc.tensor.matmul(
                            ops[:],
                            yn_h[hl][:, mt * TK : (mt + 1) * TK],
                            wpr_sb[:, hl, n * TQ : (n + 1) * TQ],
                            start=(hl == 0),
                            stop=(hl == HPC - 1),
                        )
                    osl = slice(n * TQ, (n + 1) * TQ)
                    if n == 0:
                        nc.scalar.copy(osb[:, osl], ops[:])
                    else:
                        with nc.allow_low_precision(reason="out evac to fp16"):
                            nc.vector.tensor_copy(osb[:, osl], ops[:])
                oq = ENGQ[m % 3] if qrot else nc.sync
                oq.dma_start(
                    out[b * T + mt * TK : b * T + (mt + 1) * TK, :], osb[:]
                )

        deferred = None
        for b in range(B):
            if b == 0:
                qk_tiles, backlog = qkv_cold(xt_sb)
            else:
                qk_tiles, backlog = qkv_warm(xt_sb, deferred)
                deferred = None
            v_sb = v_phase(xt_sb, qk_tiles, backlog)
            def prefetch_xt(b, cts):
                # gpsimd queue: the sync queue must stay clear for the
                # c_proj output DMAs (osb ring reuse waits on them)
                for ct in cts:
                    nc.gpsimd.dma_start(
                        xt_sb[:, ct, :],
                        xT[ct * 128 : (ct + 1) * 128, (b + 1) * T : (b + 2) * T],
                    )

            if b + 1 < B:
                xt_sb = xt_pool.tile([128, NCT, T], F16, tag="xt")
                prefetch_xt(b, range(8))
            yn_h = [
                yn_pool.tile([128, T], F16, tag=f"yn{hl}", name=f"ynt{hl}")
                for hl in range(HPC)
            ]
            # per query tile g: core(h0,g) -> cproj(g-1) -> core(h1,g);
            # c_proj lags one tile so each tile's normalize chain finishes
            # during the next tile's matmul work
            for g in range(NTQ):
                attn_core(0, g, qk_tiles, v_sb, yn_h[0])
                if g > 0:
                    emit_cproj_group(b, g - 1, yn_h)
                attn_core(1, g, qk_tiles, v_sb, yn_h[1])
                if g == 2 and b + 1 < B:
                    # second half of the prefetch, after the cycle-2 gp ops
                    # so the gpsimd SWDGE bursts never starve them
                    prefetch_xt(b, range(8, NCT))
            if b + 1 < B:
                # defer the last c_proj group into the next batch's QKV so
                # its normalize chain hides under the first mm group
                deferred = (
                    lambda b=b, yn_h=yn_h: emit_cproj_group(b, NTQ - 1, yn_h)
                )
            else:
                # final group: spread the output DMAs across queues so the
                # drain after the last matmul runs in parallel
                emit_cproj_group(b, NTQ - 1, yn_h, qrot=True)

    nc.compile()
    return nc


# ---- host-side sharding / unsharding ----

def _rope_cos_sin():
    inv_freq = 1.0 / (ROPE_BASE ** (np.arange(0, D, 2, dtype=np.float32) / D))
    t = np.arange(T, dtype=np.float32)
    freqs = np.outer(t, inv_freq).astype(np.float32)
    emb = np.concatenate([freqs, freqs], axis=-1)
    return np.cos(emb).astype(np.float32), np.sin(emb).astype(np.float32)


def _tri():
    # tri[k, q] = 1 where k <= q (valid corner of a diagonal key block)
    a = np.arange(TK)
    return (a[:, None] <= a[None, :]).astype(np.float16)


def _rot_mat():
    r = np.zeros((D, D), dtype=np.float32)  # RT: rot = (RT.T) @ q
    h = D // 2
    for d in range(h):
        r[d + h, d] = -1.0
    for d in range(h, D):
        r[d - h, d] = 1.0
    return r


_PROGRAM_CACHE = {}


def _get_program(with_bias_qk, with_bias_v):
    key = (with_bias_qk, with_bias_v)
    if key not in _PROGRAM_CACHE:
        _PROGRAM_CACHE[key] = _build_program(with_bias_qk, with_bias_v)
    return _PROGRAM_CACHE[key]


def _make_in_maps(x, W_attn, b_attn, W_proj):
    f16 = np.float16
    x = np.asarray(x, dtype=np.float32)
    W_attn = np.asarray(W_attn, dtype=np.float32)
    b_attn = np.asarray(b_attn, dtype=np.float32)
    W_proj = np.asarray(W_proj, dtype=np.float32)

    xT = np.ascontiguousarray(
        x.transpose(2, 0, 1).reshape(C, BT)
    ).astype(f16)
    Wq, Wk, Wv = W_attn[:, :C], W_attn[:, C : 2 * C], W_attn[:, 2 * C :]
    bq, bk, bvv = b_attn[:C], b_attn[C : 2 * C], b_attn[2 * C :]
    cos, sin = _rope_cos_sin()
    cosT = np.ascontiguousarray(cos.T).astype(f16)
    sinT = np.ascontiguousarray(sin.T).astype(f16)
    tri = _tri()
    rmat = _rot_mat().astype(f16)

    in_maps = []
    for c in range(N_CORES):
        h0, h1 = HPC * c, HPC * c + 1
        sl0, sl1 = slice(h0 * D, (h0 + 1) * D), slice(h1 * D, (h1 + 1) * D)
        wqk_c = np.concatenate(
            [Wq[:, sl0], Wq[:, sl1], Wk[:, sl0], Wk[:, sl1]], axis=1
        ).astype(f16).reshape(NCT, 128, 4 * D).transpose(1, 0, 2)
        wv_c = (np.concatenate([Wv[:, sl0], Wv[:, sl1]], axis=1)
                .astype(f16).reshape(NCT, 128, HPC * D).transpose(1, 0, 2))
        wpr_c = (np.concatenate([W_proj[sl0, :], W_proj[sl1, :]], axis=0)
                 .astype(f16).reshape(HPC, 128, C).transpose(1, 0, 2))
        bqk_c = np.concatenate([bq[sl0], bq[sl1], bk[sl0], bk[sl1]]).astype(
            np.float32
        ).reshape(4, 128).T
        bv_c = np.concatenate([bvv[sl0], bvv[sl1]]).astype(np.float32)
        in_maps.append(
            {
                "xT": xT,
                "wqk": np.ascontiguousarray(wqk_c),
                "wv": np.ascontiguousarray(wv_c),
                "wpr": np.ascontiguousarray(wpr_c),
                "bqk": np.ascontiguousarray(bqk_c),
                "bv": bv_c,
                "cosT": cosT,
                "sinT": sinT,
                "tri": tri,
                "rmat": rmat,
            }
        )
    return in_maps


def _ensure_ntff_hook():
    """Bridge the missing antenv.axon_hooks module so trace=True can profile.

    The axon boot code registers an NTFF profiling hook via
    antenv.axon_hooks, which this image's antenv package lacks. Install a
    minimal in-memory module and register the ctypes-based hook from
    trn_agent_boot. Only used for profiling runs; best-effort.
    """
    import types

    if "antenv.axon_hooks" in sys.modules:
        return
    try:
        import antenv

        mod = types.ModuleType("antenv.axon_hooks")
        holder = {"hook": None}
        mod.set_axon_ntff_profile_hook = lambda h: holder.__setitem__("hook", h)
        mod.get_axon_ntff_profile_hook = lambda: holder["hook"]
        sys.modules["antenv.axon_hooks"] = mod
        antenv.axon_hooks = mod
        axon_site = "/root/.axon_site"
        if os.path.isdir(axon_site) and axon_site not in sys.path:
            sys.path.insert(0, axon_site)
        from trn_agent_boot.trn_boot import _ntff_profile_via_ctypes

        hook = _ntff_profile_via_ctypes("/opt/axon/libaxon_pjrt.so")
        if hook is not None:
            mod.set_axon_ntff_profile_hook(hook)
    except Exception as e:  # profiling is best-effort
        print(f"[ntff hook unavailable: {type(e).__name__}: {e}]", flush=True)


def run(x, W_attn, b_attn, W_proj, b_proj, trace=False):
    if trace:
        _ensure_ntff_hook()
        import concourse.bass_utils as _bu

        _bu.upload_artifacts = lambda tmpdir: f"local://{tmpdir}"
    b_attn = np.asarray(b_attn, dtype=np.float32)
    b_proj = np.asarray(b_proj, dtype=np.float32)
    with_bias_qk = bool(np.any(b_attn[: 2 * C] != 0.0))
    with_bias_v = bool(np.any(b_attn[2 * C :] != 0.0))
    nc = _get_program(with_bias_qk, with_bias_v)
    in_maps = _make_in_maps(x, W_attn, b_attn, W_proj)
    res = run_bass_kernel_spmd(
        nc, in_maps, list(range(N_CORES)), trace=trace
    )
    acc = np.zeros((BT, C), dtype=np.float32)
    for r in res.results:
        acc += np.asarray(r["out"], dtype=np.float32)
    acc += b_proj[None, :]
    return acc.reshape(B, T, C).astype(np.float32), res


def kernel(x, W_attn, b_attn, W_proj, b_proj):
    out, _ = run(x, W_attn, b_attn, W_proj, b_proj, trace=False)
    return out
